# revision 2
# baseline (speedup 1.0000x reference)
"""DialogueGCN forward pass as a distributed Bass/Tile kernel on 8 TRN2 NeuronCores.

Math (reference): Bahdanau attention over utterance pairs -> per-edge softmax
weights; RGCN with per-relation weights W_rel[etype] + root term; GraphConv.

Key structural facts exploited:
  * etype = 2*(sp[i]*48 + sp[j]) + (i>=j) with speaker in {0,1} -> at most 8
    distinct relation types out of R=4608 are live. Only those 8 slices of the
    1.2GB W_rel are ever touched (host routes them to the devices).
  * The graph is fully connected, so the GraphConv neighbor sum is the same
    vector for every node: sum_i h_i.
  * agg = sum_r (attn*mask_r)^T (x @ W_r): 8 masked matmuls instead of a
    2304-edge gather/scatter.

Sharding: the RGCN/GraphConv hidden dim H=256 is split 8 ways (32 columns per
core); every core computes the full [48,48] attention (cheap, avoids a second
collective), its h-slice of the RGCN, then one AllGather of the [32,48] slices
rebuilds h^T [256,48] on every core, and each core finishes its g-slice of the
GraphConv output. Host concatenates the 8 [32,48] outputs and transposes.
"""
import numpy as np

L = 48
D = 256
H = 256
G = 256
A = 128
N_CORES = 8
HS = H // N_CORES  # 32 columns of h (and of the output) per core
NREL = 8

_compiled = None


def _build():
    """Build + schedule the Bass program once; returns (nc, input names)."""
    import concourse.bacc as bacc
    import concourse.mybir as mybir
    import concourse.tile as tile

    dt = mybir.dt.float32
    nc = bacc.Bacc("TRN2", debug=False, num_devices=N_CORES)

    # --- DRAM I/O (per-core views; replicated tensors get identical data) ---
    xt_d = nc.dram_tensor("xt", [2, 128, L], dt, kind="ExternalInput")
    wq_d = nc.dram_tensor("wq", [2, 128, A], dt, kind="ExternalInput")
    wk_d = nc.dram_tensor("wk", [2, 128, A], dt, kind="ExternalInput")
    vv_d = nc.dram_tensor("vv", [128, 1], dt, kind="ExternalInput")
    wrel_d = nc.dram_tensor("wrel", [2, 128, NREL * HS], dt, kind="ExternalInput")
    wroot_d = nc.dram_tensor("wroot", [2, 128, HS], dt, kind="ExternalInput")
    wself_d = nc.dram_tensor("wself", [2, 128, HS], dt, kind="ExternalInput")
    wnbr_d = nc.dram_tensor("wnbr", [2, 128, HS], dt, kind="ExternalInput")
    mask_d = nc.dram_tensor("maskw", [L, NREL * L], dt, kind="ExternalInput")
    brg_d = nc.dram_tensor("brg", [HS, 1], dt, kind="ExternalInput")
    bgc_d = nc.dram_tensor("bgc", [HS, 1], dt, kind="ExternalInput")
    y_d = nc.dram_tensor("yout", [HS, L], dt, kind="ExternalOutput")

    with tile.TileContext(nc) as tc:
        with (
            tc.tile_pool(name="sbuf", bufs=1) as pool,
            tc.tile_pool(name="psum", bufs=1, space="PSUM") as psum,
            tc.tile_pool(name="dram", bufs=1, space="DRAM") as dram,
        ):
            # ---- load everything (Tile spreads DMAs over HW queues) ----
            xt = [pool.tile([128, L], dt, name=f"xt{t}", tag=f"xt{t}") for t in range(2)]
            wq = [pool.tile([128, A], dt, name=f"wq{t}", tag=f"wq{t}") for t in range(2)]
            wk = [pool.tile([128, A], dt, name=f"wk{t}", tag=f"wk{t}") for t in range(2)]
            wr = [pool.tile([128, NREL * HS], dt, name=f"wr{t}", tag=f"wr{t}") for t in range(2)]
            wro = [pool.tile([128, HS], dt, name=f"wro{t}", tag=f"wro{t}") for t in range(2)]
            ws = [pool.tile([128, HS], dt, name=f"ws{t}", tag=f"ws{t}") for t in range(2)]
            wn = [pool.tile([128, HS], dt, name=f"wn{t}", tag=f"wn{t}") for t in range(2)]
            for t in range(2):
                nc.sync.dma_start(xt[t][:], xt_d.ap()[t])
                nc.sync.dma_start(wq[t][:], wq_d.ap()[t])
                nc.sync.dma_start(wk[t][:], wk_d.ap()[t])
                nc.sync.dma_start(wr[t][:], wrel_d.ap()[t])
                nc.sync.dma_start(wro[t][:], wroot_d.ap()[t])
                nc.sync.dma_start(ws[t][:], wself_d.ap()[t])
                nc.sync.dma_start(wn[t][:], wnbr_d.ap()[t])
            vv = pool.tile([128, 1], dt)
            maskt = pool.tile([L, NREL, L], dt)
            brg = pool.tile([HS, 1], dt)
            bgc = pool.tile([HS, 1], dt)
            nc.sync.dma_start(vv[:], vv_d.ap())
            nc.sync.dma_start(maskt[:], mask_d.ap().rearrange("i (r j) -> i r j", r=NREL))
            nc.sync.dma_start(brg[:], brg_d.ap())
            nc.sync.dma_start(bgc[:], bgc_d.ap())

            # ---- Bahdanau attention: scores[i,j] = v . tanh(qT[:,i]+kT[:,j]) ----
            qT_ps = psum.tile([128, L], dt, tag="attn_ps")
            for t in range(2):
                nc.tensor.matmul(qT_ps[:], wq[t][:], xt[t][:],
                                 start=(t == 0), stop=(t == 1))
            qTs = pool.tile([128, L, 1], dt)
            nc.vector.tensor_copy(qTs[:, :, 0], qT_ps[:])
            kT_ps = psum.tile([128, L], dt, tag="attn_ps")
            for t in range(2):
                nc.tensor.matmul(kT_ps[:], wk[t][:], xt[t][:],
                                 start=(t == 0), stop=(t == 1))
            kTs = pool.tile([128, 1, L], dt)
            nc.vector.tensor_copy(kTs[:, 0, :], kT_ps[:])

            bigT = pool.tile([128, L, L], dt)
            nc.vector.tensor_tensor(
                bigT[:],
                qTs[:].broadcast_to([128, L, L]),
                kTs[:].broadcast_to([128, L, L]),
                op=mybir.AluOpType.add,
            )
            tanhT = pool.tile([128, L * L], dt)
            nc.scalar.activation(tanhT[:], bigT[:].rearrange("p i j -> p (i j)"),
                                 mybir.ActivationFunctionType.Tanh)

            scores_ps = psum.tile([1, L * L], dt, tag="attn_big")
            for k in range(0, L * L, 512):
                hi = min(k + 512, L * L)
                nc.tensor.matmul(scores_ps[:, k:hi], vv[:], tanhT[:, k:hi],
                                 start=True, stop=True)
            scores_row = pool.tile([1, L * L], dt)
            nc.vector.tensor_copy(scores_row[:], scores_ps[:])
            # reshape [1, 2304] -> [48, 48] has to cross partitions: DRAM bounce
            scores_dram = dram.tile([1, L * L], dt)
            nc.sync.dma_start(scores_dram[:], scores_row[:])
            scores = pool.tile([L, L], dt)
            nc.sync.dma_start(scores[:],
                              scores_dram[:].rearrange("p (i j) -> (p i) j", i=L))

            # ---- softmax over j (scores bounded by sum|v| ~ 9: no max pass) ----
            expS = pool.tile([L, L], dt)
            rowsum = pool.tile([L, 1], dt)
            nc.scalar.activation(expS[:], scores[:], mybir.ActivationFunctionType.Exp,
                                 accum_out=rowsum[:])
            recip = pool.tile([L, 1], dt)
            nc.vector.reciprocal(recip[:], rowsum[:])
            # A_r[i,j] = (exp * 1/rowsum) * mask_r, all 8 relations in one op
            attnW = pool.tile([L, NREL, L], dt)
            nc.vector.scalar_tensor_tensor(
                attnW[:],
                expS[:].rearrange("i (o j) -> i o j", o=1).broadcast_to([L, NREL, L]),
                recip[:],
                maskt[:],
                op0=mybir.AluOpType.mult,
                op1=mybir.AluOpType.mult,
            )

            # ---- RGCN (h-slice): Yall[i, r*32+h'] = (x @ W_r[:, hsl]) ----
            yall_ps = psum.tile([L, NREL * HS], dt, tag="mm_ps")
            for t in range(2):
                nc.tensor.matmul(yall_ps[:], xt[t][:], wr[t][:],
                                 start=(t == 0), stop=(t == 1))
            yall = pool.tile([L, NREL * HS], dt)
            nc.vector.tensor_copy(yall[:], yall_ps[:])

            # hT_slice[h', j] = sum_r sum_i Y_r[i,h'] A_r[i,j] + (x@W_root)^T + b
            h_ps = psum.tile([HS, L], dt, tag="mm_ps")
            for r in range(NREL):
                nc.tensor.matmul(h_ps[:], yall[:, r * HS:(r + 1) * HS],
                                 attnW[:, r, :], start=(r == 0), stop=False)
            for t in range(2):
                nc.tensor.matmul(h_ps[:], wro[t][:], xt[t][:],
                                 start=False, stop=(t == 1))
            hTs = pool.tile([HS, L], dt)
            nc.vector.tensor_scalar_add(hTs[:], h_ps[:], brg[:])

            # ---- AllGather h-slices -> full hT [256, 48] on every core ----
            ag_in = dram.tile([HS, L], dt)
            ag_out = dram.tile([H, L], dt)
            nc.sync.dma_start(ag_in[:], hTs[:])
            nc.gpsimd.collective_compute(
                "AllGather",
                mybir.AluOpType.bypass,
                replica_groups=[list(range(N_CORES))],
                ins=[ag_in.opt()],
                outs=[ag_out.opt()],
            )
            hfull = pool.tile([128, 2 * L], dt)
            agv = ag_out[:].rearrange("(t p) f -> t p f", p=128)
            for t in range(2):
                nc.sync.dma_start(hfull[:, t * L:(t + 1) * L], agv[t])

            # ---- GraphConv (g-slice): out^T = W_self^T hT + (W_nbr^T s + b) ----
            sT = pool.tile([128, 2], dt)
            for t in range(2):
                nc.vector.reduce_sum(sT[:, t:t + 1], hfull[:, t * L:(t + 1) * L],
                                     axis=mybir.AxisListType.X)
            nb_ps = psum.tile([HS, 1], dt, tag="nb_ps")
            for t in range(2):
                nc.tensor.matmul(nb_ps[:], wn[t][:], sT[:, t:t + 1],
                                 start=(t == 0), stop=(t == 1))
            nbs = pool.tile([HS, 1], dt)
            nc.vector.tensor_scalar_add(nbs[:], nb_ps[:], bgc[:])

            out_ps = psum.tile([HS, L], dt, tag="mm_ps")
            for t in range(2):
                nc.tensor.matmul(out_ps[:], ws[t][:], hfull[:, t * L:(t + 1) * L],
                                 start=(t == 0), stop=(t == 1))
            outs = pool.tile([HS, L], dt)
            nc.vector.tensor_scalar_add(outs[:], out_ps[:], nbs[:])
            nc.sync.dma_start(y_d.ap(), outs[:])

    nc.compile()
    return nc


def _prepare_in_maps(global_features, speaker, Wq, Wk, v, W_rel, W_root, b_rgcn,
                     W_nbr, W_self, b_gcn):
    """Host-side routing: pick the <=8 live relation slices, build masks, pack
    per-core shards (h-slice of RGCN weights, g-slice of GraphConv weights)."""
    f32 = np.float32
    x = np.ascontiguousarray(global_features, dtype=f32)
    sp = np.asarray(speaker).astype(np.int64)
    n = L

    ii, jj = np.meshgrid(np.arange(n), np.arange(n), indexing="ij")
    direction = (ii >= jj).astype(np.int64)
    et = 2 * (sp[ii] * n + sp[jj]) + direction  # [48, 48] edge-type grid

    rel_ids = np.unique(et)
    assert len(rel_ids) <= NREL, f"{len(rel_ids)} live relations > {NREL}"
    masks = np.zeros((NREL, n, n), dtype=f32)
    rel_pad = np.full(NREL, rel_ids[0], dtype=np.int64)
    for s, rid in enumerate(rel_ids):
        masks[s] = (et == rid)
        rel_pad[s] = rid
    # padded slots keep zero masks -> contribute nothing

    W_used = np.ascontiguousarray(np.asarray(W_rel)[rel_pad], dtype=f32)  # [8,256,256]

    xt = np.ascontiguousarray(x.T).reshape(2, 128, L)
    wq = np.ascontiguousarray(Wq, dtype=f32).reshape(2, 128, A)
    wk = np.ascontiguousarray(Wk, dtype=f32).reshape(2, 128, A)
    vv = np.ascontiguousarray(v, dtype=f32).reshape(128, 1)
    maskw = np.ascontiguousarray(masks.transpose(1, 0, 2)).reshape(L, NREL * L)
    W_root = np.asarray(W_root, dtype=f32)
    W_self = np.asarray(W_self, dtype=f32)
    W_nbr = np.asarray(W_nbr, dtype=f32)
    b_rgcn = np.asarray(b_rgcn, dtype=f32)
    b_gcn = np.asarray(b_gcn, dtype=f32)

    in_maps = []
    for c in range(N_CORES):
        sl = slice(c * HS, (c + 1) * HS)
        wrel_c = np.ascontiguousarray(
            W_used[:, :, sl].transpose(1, 0, 2)).reshape(2, 128, NREL * HS)
        in_maps.append({
            "xt": xt, "wq": wq, "wk": wk, "vv": vv, "maskw": maskw,
            "wrel": wrel_c,
            "wroot": np.ascontiguousarray(W_root[:, sl]).reshape(2, 128, HS),
            "wself": np.ascontiguousarray(W_self[:, sl]).reshape(2, 128, HS),
            "wnbr": np.ascontiguousarray(W_nbr[:, sl]).reshape(2, 128, HS),
            "brg": np.ascontiguousarray(b_rgcn[sl]).reshape(HS, 1),
            "bgc": np.ascontiguousarray(b_gcn[sl]).reshape(HS, 1),
        })
    return in_maps


def kernel(global_features, speaker, Wq, Wk, v, W_rel, W_root, b_rgcn,
           W_nbr, W_self, b_gcn):
    global _compiled
    from concourse.bass_utils import run_bass_kernel_spmd

    if _compiled is None:
        _compiled = _build()
    nc = _compiled

    in_maps = _prepare_in_maps(global_features, speaker, Wq, Wk, v, W_rel,
                               W_root, b_rgcn, W_nbr, W_self, b_gcn)
    res = run_bass_kernel_spmd(nc, in_maps, core_ids=list(range(N_CORES)))
    outT = np.concatenate([res.results[c]["yout"] for c in range(N_CORES)], axis=0)
    return np.ascontiguousarray(outT.T)


# revision 3
# speedup vs baseline: 16.0002x; 16.0002x over previous
"""DialogueGCN forward pass as a distributed Bass/Tile kernel on 8 TRN2 NeuronCores.

Math (reference): Bahdanau attention over utterance pairs -> per-edge softmax
weights; RGCN with per-relation weights W_rel[etype] + root term; GraphConv.

Key structural facts exploited:
  * etype = 2*(sp[i]*48 + sp[j]) + (i>=j) with speaker in {0,1} -> at most 8
    distinct relation types out of R=4608 are live. Only those 8 slices of the
    1.2GB W_rel are ever touched (host routes them to the devices).
  * The graph is fully connected, so the GraphConv neighbor sum is the same
    vector for every node: sum_i h_i.
  * agg = sum_r (attn*mask_r)^T (x @ W_r): 8 masked matmuls instead of a
    2304-edge gather/scatter.

Sharding: the RGCN/GraphConv hidden dim H=256 is split 8 ways (32 columns per
core); every core computes the full [48,48] attention (cheap, avoids a second
collective), its h-slice of the RGCN, then one AllGather of the [32,48] slices
rebuilds h^T [256,48] on every core, and each core finishes its g-slice of the
GraphConv output. Host concatenates the 8 [32,48] outputs and transposes.
"""
import numpy as np

L = 48
D = 256
H = 256
G = 256
A = 128
N_CORES = 8
HS = H // N_CORES  # 32 columns of h (and of the output) per core
NREL = 8

_compiled = None


def _emit_body(nc, mybir, pool, psum, dram, d, rep, collective, n_cores):
    """Emit one full forward pass. `d` maps dram-tensor names to handles."""
    dt = mybir.dt.float32
    u = f"_{rep}"

    # ---- load everything (Tile spreads DMAs over HW queues) ----
    xt = [pool.tile([128, L], dt, name=f"xt{t}{u}", tag=f"xt{t}") for t in range(2)]
    wq = [pool.tile([128, A], dt, name=f"wq{t}{u}", tag=f"wq{t}") for t in range(2)]
    wk = [pool.tile([128, A], dt, name=f"wk{t}{u}", tag=f"wk{t}") for t in range(2)]
    wr = [pool.tile([128, NREL * HS], dt, name=f"wr{t}{u}", tag=f"wr{t}")
          for t in range(2)]
    wro = [pool.tile([128, HS], dt, name=f"wro{t}{u}", tag=f"wro{t}") for t in range(2)]
    ws = [pool.tile([128, HS], dt, name=f"ws{t}{u}", tag=f"ws{t}") for t in range(2)]
    wn = [pool.tile([128, HS], dt, name=f"wn{t}{u}", tag=f"wn{t}") for t in range(2)]
    for t in range(2):
        nc.sync.dma_start(xt[t][:], d["xt"].ap()[t])
        nc.sync.dma_start(wq[t][:], d["wq"].ap()[t])
        nc.sync.dma_start(wk[t][:], d["wk"].ap()[t])
        nc.sync.dma_start(wr[t][:], d["wrel"].ap()[t])
        nc.sync.dma_start(wro[t][:], d["wroot"].ap()[t])
        nc.sync.dma_start(ws[t][:], d["wself"].ap()[t])
        nc.sync.dma_start(wn[t][:], d["wnbr"].ap()[t])
    vv = pool.tile([128, 1], dt, name=f"vv{u}", tag="vv")
    maskt = pool.tile([L, NREL, L], dt, name=f"maskt{u}", tag="maskt")
    brg = pool.tile([HS, 1], dt, name=f"brg{u}", tag="brg")
    bgc = pool.tile([HS, 1], dt, name=f"bgc{u}", tag="bgc")
    nc.sync.dma_start(vv[:], d["vv"].ap())
    nc.sync.dma_start(maskt[:], d["maskw"].ap().rearrange("i (r j) -> i r j", r=NREL))
    nc.sync.dma_start(brg[:], d["brg"].ap())
    nc.sync.dma_start(bgc[:], d["bgc"].ap())

    # ---- Bahdanau attention: scores[i,j] = v . tanh(qT[:,i]+kT[:,j]) ----
    qT_ps = psum.tile([128, L], dt, name=f"qT_ps{u}", tag="attn_ps")
    for t in range(2):
        nc.tensor.matmul(qT_ps[:], wq[t][:], xt[t][:], start=(t == 0), stop=(t == 1))
    qTs = pool.tile([128, L, 1], dt, name=f"qTs{u}", tag="qTs")
    nc.vector.tensor_copy(qTs[:, :, 0], qT_ps[:])
    kT_ps = psum.tile([128, L], dt, name=f"kT_ps{u}", tag="attn_ps")
    for t in range(2):
        nc.tensor.matmul(kT_ps[:], wk[t][:], xt[t][:], start=(t == 0), stop=(t == 1))
    kTs = pool.tile([128, 1, L], dt, name=f"kTs{u}", tag="kTs")
    nc.vector.tensor_copy(kTs[:, 0, :], kT_ps[:])

    bigT = pool.tile([128, L, L], dt, name=f"bigT{u}", tag="bigT")
    nc.vector.tensor_tensor(
        bigT[:],
        qTs[:].broadcast_to([128, L, L]),
        kTs[:].broadcast_to([128, L, L]),
        op=mybir.AluOpType.add,
    )
    tanhT = pool.tile([128, L * L], dt, name=f"tanhT{u}", tag="tanhT")
    nc.scalar.activation(tanhT[:], bigT[:].rearrange("p i j -> p (i j)"),
                         mybir.ActivationFunctionType.Tanh)

    scores_ps = psum.tile([1, L * L], dt, name=f"scores_ps{u}", tag="attn_big")
    for k in range(0, L * L, 512):
        hi = min(k + 512, L * L)
        nc.tensor.matmul(scores_ps[:, k:hi], vv[:], tanhT[:, k:hi],
                         start=True, stop=True)
    scores_row = pool.tile([1, L * L], dt, name=f"scores_row{u}", tag="scores_row")
    nc.vector.tensor_copy(scores_row[:], scores_ps[:])
    # reshape [1, 2304] -> [48, 48] has to cross partitions: DRAM bounce
    scores_dram = dram.tile([1, L * L], dt, name=f"scores_dram{u}", tag="scores_dram")
    nc.sync.dma_start(scores_dram[:], scores_row[:])
    scores = pool.tile([L, L], dt, name=f"scores{u}", tag="scores")
    nc.sync.dma_start(scores[:], scores_dram[:].rearrange("p (i j) -> (p i) j", i=L))

    # ---- softmax over j (scores bounded by sum|v| ~ 9: no max pass) ----
    expS = pool.tile([L, L], dt, name=f"expS{u}", tag="expS")
    rowsum = pool.tile([L, 1], dt, name=f"rowsum{u}", tag="rowsum")
    nc.scalar.activation(expS[:], scores[:], mybir.ActivationFunctionType.Exp,
                         accum_out=rowsum[:])
    recip = pool.tile([L, 1], dt, name=f"recip{u}", tag="recip")
    nc.vector.reciprocal(recip[:], rowsum[:])
    # A_r[i,j] = (exp * 1/rowsum) * mask_r, all 8 relations in one op
    attnW = pool.tile([L, NREL, L], dt, name=f"attnW{u}", tag="attnW")
    nc.vector.scalar_tensor_tensor(
        attnW[:],
        expS[:].rearrange("i (o j) -> i o j", o=1).broadcast_to([L, NREL, L]),
        recip[:],
        maskt[:],
        op0=mybir.AluOpType.mult,
        op1=mybir.AluOpType.mult,
    )

    # ---- RGCN (h-slice): Yall[i, r*32+h'] = (x @ W_r[:, hsl]) ----
    yall_ps = psum.tile([L, NREL * HS], dt, name=f"yall_ps{u}", tag="mm_ps")
    for t in range(2):
        nc.tensor.matmul(yall_ps[:], xt[t][:], wr[t][:], start=(t == 0), stop=(t == 1))
    yall = pool.tile([L, NREL * HS], dt, name=f"yall{u}", tag="yall")
    nc.vector.tensor_copy(yall[:], yall_ps[:])

    # hT_slice[h', j] = sum_r sum_i Y_r[i,h'] A_r[i,j] + (x@W_root)^T + b
    h_ps = psum.tile([HS, L], dt, name=f"h_ps{u}", tag="mm_ps")
    for r in range(NREL):
        nc.tensor.matmul(h_ps[:], yall[:, r * HS:(r + 1) * HS], attnW[:, r, :],
                         start=(r == 0), stop=False)
    for t in range(2):
        nc.tensor.matmul(h_ps[:], wro[t][:], xt[t][:], start=False, stop=(t == 1))
    hTs = pool.tile([HS, L], dt, name=f"hTs{u}", tag="hTs")
    nc.vector.tensor_scalar_add(hTs[:], h_ps[:], brg[:])

    # ---- AllGather h-slices -> full hT [256, 48] on every core ----
    ag_in = dram.tile([HS, L], dt, name=f"ag_in{u}", tag="ag_in")
    ag_out = dram.tile([H, L], dt, name=f"ag_out{u}", tag="ag_out")
    nc.sync.dma_start(ag_in[:], hTs[:])
    if collective:
        nc.gpsimd.collective_compute(
            "AllGather",
            mybir.AluOpType.bypass,
            replica_groups=[list(range(n_cores))],
            ins=[ag_in.opt()],
            outs=[ag_out.opt()],
        )
    else:
        # single-core stand-in for TimelineSim: replicate the slice 8x
        agw = ag_out[:].rearrange("(c p) f -> c p f", p=HS)
        for c in range(N_CORES):
            nc.sync.dma_start(agw[c], ag_in[:])
    hfull = pool.tile([128, 2 * L], dt, name=f"hfull{u}", tag="hfull")
    agv = ag_out[:].rearrange("(t p) f -> t p f", p=128)
    for t in range(2):
        nc.sync.dma_start(hfull[:, t * L:(t + 1) * L], agv[t])

    # ---- GraphConv (g-slice): out^T = W_self^T hT + (W_nbr^T s + b) ----
    sT = pool.tile([128, 2], dt, name=f"sT{u}", tag="sT")
    for t in range(2):
        nc.vector.reduce_sum(sT[:, t:t + 1], hfull[:, t * L:(t + 1) * L],
                             axis=mybir.AxisListType.X)
    nb_ps = psum.tile([HS, 1], dt, name=f"nb_ps{u}", tag="nb_ps")
    for t in range(2):
        nc.tensor.matmul(nb_ps[:], wn[t][:], sT[:, t:t + 1],
                         start=(t == 0), stop=(t == 1))
    nbs = pool.tile([HS, 1], dt, name=f"nbs{u}", tag="nbs")
    nc.vector.tensor_scalar_add(nbs[:], nb_ps[:], bgc[:])

    out_ps = psum.tile([HS, L], dt, name=f"out_ps{u}", tag="mm_ps")
    for t in range(2):
        nc.tensor.matmul(out_ps[:], ws[t][:], hfull[:, t * L:(t + 1) * L],
                         start=(t == 0), stop=(t == 1))
    outs = pool.tile([HS, L], dt, name=f"outs{u}", tag="outs")
    nc.vector.tensor_scalar_add(outs[:], out_ps[:], nbs[:])
    nc.sync.dma_start(d["yout"].ap(), outs[:])


def build_program(n_cores=N_CORES, collective=True, repeat=1):
    """Build + schedule + compile the Bass program."""
    import concourse.bacc as bacc
    import concourse.mybir as mybir
    import concourse.tile as tile

    dt = mybir.dt.float32
    nc = bacc.Bacc("TRN2", debug=False, num_devices=n_cores)

    d = {}
    d["xt"] = nc.dram_tensor("xt", [2, 128, L], dt, kind="ExternalInput")
    d["wq"] = nc.dram_tensor("wq", [2, 128, A], dt, kind="ExternalInput")
    d["wk"] = nc.dram_tensor("wk", [2, 128, A], dt, kind="ExternalInput")
    d["vv"] = nc.dram_tensor("vv", [128, 1], dt, kind="ExternalInput")
    d["wrel"] = nc.dram_tensor("wrel", [2, 128, NREL * HS], dt, kind="ExternalInput")
    d["wroot"] = nc.dram_tensor("wroot", [2, 128, HS], dt, kind="ExternalInput")
    d["wself"] = nc.dram_tensor("wself", [2, 128, HS], dt, kind="ExternalInput")
    d["wnbr"] = nc.dram_tensor("wnbr", [2, 128, HS], dt, kind="ExternalInput")
    d["maskw"] = nc.dram_tensor("maskw", [L, NREL * L], dt, kind="ExternalInput")
    d["brg"] = nc.dram_tensor("brg", [HS, 1], dt, kind="ExternalInput")
    d["bgc"] = nc.dram_tensor("bgc", [HS, 1], dt, kind="ExternalInput")
    d["yout"] = nc.dram_tensor("yout", [HS, L], dt, kind="ExternalOutput")

    with tile.TileContext(nc) as tc:
        with (
            tc.tile_pool(name="sbuf", bufs=1) as pool,
            tc.tile_pool(name="psum", bufs=1, space="PSUM") as psum,
            tc.tile_pool(name="dram", bufs=1, space="DRAM") as dram,
        ):
            for rep in range(repeat):
                _emit_body(nc, mybir, pool, psum, dram, d, rep, collective, n_cores)

    nc.compile()
    return nc


def _prepare_in_maps(global_features, speaker, Wq, Wk, v, W_rel, W_root, b_rgcn,
                     W_nbr, W_self, b_gcn):
    """Host-side routing: pick the <=8 live relation slices, build masks, pack
    per-core shards (h-slice of RGCN weights, g-slice of GraphConv weights)."""
    f32 = np.float32
    x = np.ascontiguousarray(global_features, dtype=f32)
    sp = np.asarray(speaker).astype(np.int64)
    n = L

    ii, jj = np.meshgrid(np.arange(n), np.arange(n), indexing="ij")
    direction = (ii >= jj).astype(np.int64)
    et = 2 * (sp[ii] * n + sp[jj]) + direction  # [48, 48] edge-type grid

    rel_ids = np.unique(et)
    assert len(rel_ids) <= NREL, f"{len(rel_ids)} live relations > {NREL}"
    masks = np.zeros((NREL, n, n), dtype=f32)
    rel_pad = np.full(NREL, rel_ids[0], dtype=np.int64)
    for s, rid in enumerate(rel_ids):
        masks[s] = (et == rid)
        rel_pad[s] = rid
    # padded slots keep zero masks -> contribute nothing

    W_used = np.ascontiguousarray(np.asarray(W_rel)[rel_pad], dtype=f32)  # [8,256,256]

    xt = np.ascontiguousarray(x.T).reshape(2, 128, L)
    wq = np.ascontiguousarray(Wq, dtype=f32).reshape(2, 128, A)
    wk = np.ascontiguousarray(Wk, dtype=f32).reshape(2, 128, A)
    vv = np.ascontiguousarray(v, dtype=f32).reshape(128, 1)
    maskw = np.ascontiguousarray(masks.transpose(1, 0, 2)).reshape(L, NREL * L)
    W_root = np.asarray(W_root, dtype=f32)
    W_self = np.asarray(W_self, dtype=f32)
    W_nbr = np.asarray(W_nbr, dtype=f32)
    b_rgcn = np.asarray(b_rgcn, dtype=f32)
    b_gcn = np.asarray(b_gcn, dtype=f32)

    in_maps = []
    for c in range(N_CORES):
        sl = slice(c * HS, (c + 1) * HS)
        wrel_c = np.ascontiguousarray(
            W_used[:, :, sl].transpose(1, 0, 2)).reshape(2, 128, NREL * HS)
        in_maps.append({
            "xt": xt, "wq": wq, "wk": wk, "vv": vv, "maskw": maskw,
            "wrel": wrel_c,
            "wroot": np.ascontiguousarray(W_root[:, sl]).reshape(2, 128, HS),
            "wself": np.ascontiguousarray(W_self[:, sl]).reshape(2, 128, HS),
            "wnbr": np.ascontiguousarray(W_nbr[:, sl]).reshape(2, 128, HS),
            "brg": np.ascontiguousarray(b_rgcn[sl]).reshape(HS, 1),
            "bgc": np.ascontiguousarray(b_gcn[sl]).reshape(HS, 1),
        })
    return in_maps


def kernel(global_features, speaker, Wq, Wk, v, W_rel, W_root, b_rgcn,
           W_nbr, W_self, b_gcn):
    global _compiled
    from concourse.bass_utils import run_bass_kernel_spmd

    if _compiled is None:
        _compiled = build_program()
    nc = _compiled

    in_maps = _prepare_in_maps(global_features, speaker, Wq, Wk, v, W_rel,
                               W_root, b_rgcn, W_nbr, W_self, b_gcn)
    res = run_bass_kernel_spmd(nc, in_maps, core_ids=list(range(N_CORES)))
    outT = np.concatenate([res.results[c]["yout"] for c in range(N_CORES)], axis=0)
    return np.ascontiguousarray(outT.T)
